# revision 32
# baseline (speedup 1.0000x reference)
"""Trainium2 Bass kernel for Clique2NodeConvBasic (GNN message passing).

Computes, for N=100000 nodes, C=50000 cliques, E=1600000 edges, D=128:

    gathered = x_clique[clique_idx]            # [E, 128]
    summed   = segment_sum(gathered, node_idx) # [N, 128]
    mean     = summed / max(count, 1)
    out      = mean @ W.T + b                  # [N, 128]

Sharding: edges partitioned by destination-node range across 8 NeuronCores
(12500 nodes per core); x_clique and the Linear weights replicated.

v6 design; measured ~420-421 us median (v3 was ~485, first-session
baseline 1953), rel err 2.8e-3 (gate 2e-2). These axon cores show
run-to-run variance with occasional +8-25% outliers; judge changes by
multi-run median.

THE GOVERNING LAW (microbenched via bench.py): dma_gather throughput is
bound by per-DESCRIPTOR SDMA processing, ~2.4 ns/desc with 4 SWDGE
queues (single queue 7.9; 4 = ucode MAX_SWDGE_QUEUES), and is nearly
INSENSITIVE to descriptor SIZE (512-B descs cost 2.47 ns vs 2.40 for
256-B at 24-tile calls) and to source (SBUF-source transpose gather is
SLOWER, 2.8; sorted HBM addresses identical; transpose mode identical;
single_packet identical at small calls, >64 descs/engine per packet
hangs). So wall time ~= #descriptors x 2.4ns + ramp (~20us) + matmul
tail (~25us) + teardown (~10us). The ONLY big lever is FEWER
DESCRIPTORS; bytes are nearly free.

  - PAIRED 512-B TABLE ENTRIES: per core, cliques are greedily matched
    into 25000 pairs maximizing co-residency in destination blocks
    (bucket passes: pairs sharing >=3 blocks via block-triple keys, then
    >=2 via block-pairs, then >=1, then arbitrary). The per-core HBM
    table xcP[p] = [xc[c1] | xc[c2]]. A destination block needs ONE
    descriptor per pair-ENTRY touched (not per clique): 196K dedup'd
    (block, clique) groups -> ~133K descriptors (~32% saved; S ~ 64K
    shared blocks). Pair index < 25000 also fits int16, so the old
    A/B >=32768 stream split is gone (single stream, less padding).
  - Each gathered tile is consumed by TWO matmuls per column: even half
    g[:, slot, 0:128] and odd half [:, 128:256], each with its own
    multi-hot fp8 column (host-built, multiplicity-valued; one column
    has 1s for EVERY edge of that position wanting that half).
  - One-hots are HOST-PRECOMPUTED and DMA'd (~36 MB/core on the sync
    HWDGE ring; bytes are free next to the desc-bound gather). The DVE
    is_equal generation of v3 (327us DVE busy, PORT-shared with Q7) is
    gone; DVE now only does the +bias add (the old rank-1 cnt*bias
    matmul trick is dropped -- its [1, NPAD] cnt tile wasted 24.5 KB of
    column space since [1, X] tiles pad to 128 partitions).
  - Call size NT=28 tiles (3584 slots, 1.8 MB): per-call fixed cost for
    512-B calls is ~1.26us (vs 0.63 for 256-B), so small calls hurt:
    NT=12 ran at 2.88 ns/desc vs 2.47 at NT=24. gpool/opool bufs=2
    (SBUF: g tiles 14 KB/partition/buf). Tail tapers to 8/4-tile calls
    (fewer open positions -> shorter matmul tail).
  - Same edge-granular packing as v3: positions = blocks sorted by
    per-core pair-group count, padded to the max over 8 cores, packed
    back-to-back; a tile spanning two positions is matmul'd once per
    (position, half) with position-masked columns. Host un-permutes.
  - Epilogue per position: ACT copy accum->bf16, Linear matmul
    (accum[f,n] IS the lhsT), ACT 1/max(cnt,1) scale, DVE +bias
    (host-replicated [128,128] row), DMA out.
  - Benched and rejected: ap_gather (27.7 ns/idx, also Q7), scatter_add
    (same desc rate + RMW), fp8 gathered data (elem>=256B assert; also
    bytes are free so no point), PSUM ps>6/psl>2 (bank-granular: 8
    banks total), partition-offset stationary (base must be 0/32/64),
    DVE partition-broadcast operands (zero-step assert), oh upload on
    scalar HWDGE ring (421 -> 452-515 ascending outliers), NT=20
    gpool bufs=3 (427), warm-up dummy gather (no ramp change; kept).
"""

import os
import sys
import types

sys.path.insert(0, "/opt/trn_rl_repo")

import numpy as np

import concourse.bass as bass
import concourse.mybir as mybir
import concourse.tile as tile
from concourse.vector_clock import ScopedClock, VectorClock
from concourse.bass_utils import run_bass_kernel_spmd

# ----------------------------------------------------------------------------
# Environment shims
# ----------------------------------------------------------------------------

def _install_ntff_shim():
    """Register the axon NTFF profile hook if the image's antenv lacks it."""
    try:
        import antenv
    except ImportError:
        return
    if hasattr(antenv, "axon_hooks"):
        return
    hooks_mod = types.ModuleType("antenv.axon_hooks")
    _store = [None]
    hooks_mod.set_axon_ntff_profile_hook = lambda h: _store.__setitem__(0, h)
    hooks_mod.get_axon_ntff_profile_hook = lambda: _store[0]
    sys.modules["antenv.axon_hooks"] = hooks_mod
    antenv.axon_hooks = hooks_mod
    try:
        from trn_agent_boot.trn_boot import _ntff_profile_via_ctypes

        hook = _ntff_profile_via_ctypes("/opt/axon/libaxon_pjrt.so")
        if hook is not None:
            hooks_mod.set_axon_ntff_profile_hook(hook)
    except Exception:
        pass


_install_ntff_shim()


class PatchedTileContext(tile.TileContext):
    """Spread the tail-drain's sem waits over a chain of SP NOPs.

    The walrus build in this container caps sync-waits per instruction
    (setupSyncWait: "Too many sync wait commands"), while stock Tile
    attaches every outstanding proc's wait to one Drain. One NOP per
    proc keeps every instruction at a single wait.
    """

    def _drain_and_barrier(self, tick_clock, wait_clock):
        gc = tick_clock.global_clock
        pending = [(p, t) for p, t in enumerate(gc) if t > 0]
        for i in range(0, len(pending), 8):
            nop = self.nc.sync.nop()
            part = VectorClock()
            for p, t in pending[i : i + 8]:
                part.require_at_least(p, t)
            wait_clock.add_sem_waits(nop.ins, ScopedClock({None: part}))
        self.nc.sync.drain()
        self.nc.all_engine_barrier()
        assert self.sems is not None
        popped = self.nc._tile_sem_poison_stack.pop()
        assert popped is self._sem_poison
        self.nc.clear_and_free_semaphores(list(self.sems.allocated().values()))
        self.nc.all_engine_barrier()


# ----------------------------------------------------------------------------
# Problem constants (hardcoded per the task contract)
# ----------------------------------------------------------------------------

N_NODES = 100000
N_CLIQUES = 50000
D = 128
N_CORES = 8
NPC = N_NODES // N_CORES        # 12500 nodes per core
BLK = 128                       # destination nodes per block
NBLK = -(-NPC // BLK)           # 98 blocks per core (last partial: 84)
NPAD = NBLK * BLK               # 12544 padded output rows per core
NPAIR = N_CLIQUES // 2          # 25000 paired 512-B table entries
NT = 28                         # 128-slot tiles per dma_gather call
NQ = 4                          # SWDGE queues used round-robin
OHW = 2 * (NT + 6)              # one-hot buffer columns per call (2 halves)

_F32 = mybir.dt.float32
_BF16 = mybir.dt.bfloat16
_FP8 = mybir.dt.float8e4

import ml_dtypes

_NP_BF16 = np.dtype(ml_dtypes.bfloat16)
_NP_FP8 = np.dtype(ml_dtypes.float8_e4m3)


# ----------------------------------------------------------------------------
# Host-side preparation
# ----------------------------------------------------------------------------

def _sched_stream(L):
    """Edge-granular stream schedule from per-position edge counts L.

    Positions are packed back-to-back (no per-position rounding); a 128-row
    tile may span two positions, in which case it appears in BOTH positions'
    matmul column lists (each column's dest marks only its own position's
    slots, the rest -1000)."""
    E = np.concatenate([[0], np.cumsum(L)]).astype(np.int64)
    ntile = -(-int(E[-1]) // 128)
    cols = []
    pos_cols = []
    for j in range(len(L)):
        if E[j + 1] == E[j]:
            pos_cols.append((len(cols), 0))
            continue
        t0 = int(E[j]) // 128
        t1 = (int(E[j + 1]) - 1) // 128
        pos_cols.append((len(cols), t1 - t0 + 1))
        for t in range(t0, t1 + 1):
            cols.append((j, t))
    calls = []
    t = 0
    k = 0
    # ramp-up: a full call's descriptors overflow its SWDGE ring and PARK
    # the gpsimd stream until drained (observed: call 0 drains ALONE at
    # single-queue speed for ~28us while queues 1-3 sit empty). Small
    # first calls stock all 4 rings within a few us.
    ramp = (4, 4, 4, 4, 8, 8, 8, 8)
    while t < ntile:
        rem = ntile - t
        if k < len(ramp):
            nt = min(ramp[k], rem)
        elif rem > NT + 16:
            nt = NT
        elif rem > 16:
            nt = 16
        elif rem > 8:
            nt = 8
        else:
            nt = min(4, rem)
        calls.append((t, nt))
        t += nt
        k += 1
    col_t = np.array([t for _, t in cols])
    call_cols = []
    for t0, nt in calls:
        m0 = int(np.searchsorted(col_t, t0, side="left"))
        m1 = int(np.searchsorted(col_t, t0 + nt - 1, side="right"))
        call_cols.append((m0, m1))
    tile_call = {}
    for i, (t0, nt) in enumerate(calls):
        for tt in range(t0, t0 + nt):
            tile_call[tt] = i
    return dict(E=E, ntile=ntile, cols=cols, pos_cols=pos_cols, calls=calls,
                call_cols=call_cols, tile_call=tile_call)


def _prepare(x_clique, node2clique_index):
    """Sort/bucket the edge list. Returns per-core input dicts plus the
    (data-dependent) uniform schedule.

    Edges are deduplicated per (block, clique), then cliques are PAIRED
    per core (greedy matching maximizing co-residency in destination
    blocks) into 512-B table entries [xc[c1] | xc[c2]]: one descriptor
    delivers both halves, and any block where the partners co-occur needs
    ONE descriptor for all their edges (~25% fewer descriptors). Each
    gathered tile is consumed by TWO matmuls (even/odd 128-feature half,
    each with its own multi-hot column)."""
    node = np.asarray(node2clique_index[0]).astype(np.int64)
    clique = np.asarray(node2clique_index[1]).astype(np.int64)

    counts = np.bincount(node, minlength=N_NODES).astype(np.float64)
    inv_cnt = (1.0 / np.maximum(counts, 1.0)).astype(np.float32)

    order = np.argsort(node, kind="stable")
    ns = node[order]
    cs = clique[order]
    core_bounds = np.searchsorted(ns, np.arange(N_CORES + 1) * NPC)

    def _match(g_blk, g_cq):
        """Pair cliques maximizing shared destination blocks (per core).

        g_blk/g_cq: distinct (block, clique) groups, sorted by (blk, cq).
        Returns partner[] covering every clique (arbitrary for leftovers)."""
        o = np.argsort(g_cq, kind="stable")
        sc, sb = g_cq[o], g_blk[o]
        starts = np.searchsorted(sc, np.arange(N_CLIQUES + 1))
        kcnt = np.diff(starts)
        partner = np.full(N_CLIQUES, -1, dtype=np.int64)

        def _bucket_pass(min_k, tuple_size):
            keys, owners = [], []
            for ci in np.flatnonzero((kcnt >= min_k) & (partner < 0)):
                bl = sb[starts[ci] : starts[ci + 1]]
                n = len(bl)
                if tuple_size == 3:
                    for i in range(n):
                        for j2 in range(i + 1, n):
                            for j3 in range(j2 + 1, n):
                                keys.append(
                                    (bl[i] * NBLK + bl[j2]) * NBLK + bl[j3]
                                )
                                owners.append(ci)
                elif tuple_size == 2:
                    for i in range(n):
                        for j2 in range(i + 1, n):
                            keys.append(bl[i] * NBLK + bl[j2])
                            owners.append(ci)
                else:
                    for b in bl:
                        keys.append(b)
                        owners.append(ci)
            if not keys:
                return
            keys = np.asarray(keys, dtype=np.int64)
            owners = np.asarray(owners, dtype=np.int64)
            o3 = np.argsort(keys, kind="stable")
            keys, owners = keys[o3], owners[o3]
            bnd = np.flatnonzero(np.diff(keys)) + 1
            bs = np.concatenate([[0], bnd, [len(keys)]])
            for bi in range(len(bs) - 1):
                cand = np.unique(owners[bs[bi] : bs[bi + 1]])
                free = cand[partner[cand] < 0]
                m = len(free) // 2 * 2
                if m:
                    partner[free[0:m:2]] = free[1:m:2]
                    partner[free[1:m:2]] = free[0:m:2]

        _bucket_pass(3, 3)   # pairs sharing >=3 blocks
        _bucket_pass(2, 2)   # pairs sharing >=2 blocks
        _bucket_pass(1, 1)   # leftovers sharing >=1 block
        # final: pair remaining (incl. zero-edge cliques) arbitrarily
        rest = np.flatnonzero(partner < 0)
        partner[rest[0::2]] = rest[1::2]
        partner[rest[1::2]] = rest[0::2]
        return partner

    def _wrap(idx):
        # [L] -> [128, L/16] (16-partition wrap, replicated to 8 core groups)
        w = idx.reshape(-1, 16).T.copy().reshape(16, -1)
        return np.tile(w, (8, 1))

    xc = np.asarray(x_clique)
    xc_bf = np.ascontiguousarray(xc).astype(_NP_BF16)

    # Pass 1 (per core): dedup groups, match cliques, count pair-groups.
    per_core = []
    cnt = np.zeros((N_CORES, NBLK), dtype=np.int64)
    for c in range(N_CORES):
        lo, hi = core_bounds[c], core_bounds[c + 1]
        loc = ns[lo:hi] - c * NPC
        cq = cs[lo:hi]
        blk = loc // BLK
        win = loc % BLK
        key = blk * N_CLIQUES + cq
        sub = np.argsort(key, kind="stable")
        blk, win, cq = blk[sub], win[sub], cq[sub]
        key = key[sub]
        first = np.concatenate([[True], key[1:] != key[:-1]])
        gid = np.cumsum(first) - 1          # per-edge (blk, clique) group id
        g_blk = blk[first]
        g_cq = cq[first]

        partner = _match(g_blk, g_cq)
        # pair table: entry p = (c1, c2), c1 < c2; half(c1)=0, half(c2)=1
        c1s = np.flatnonzero(np.arange(N_CLIQUES) < partner)
        perm = np.empty(2 * len(c1s), dtype=np.int64)
        perm[0::2] = c1s
        perm[1::2] = partner[c1s]
        assert len(perm) == N_CLIQUES
        pair_of = np.empty(N_CLIQUES, dtype=np.int64)
        half_of = np.empty(N_CLIQUES, dtype=np.int64)
        pair_of[perm] = np.arange(N_CLIQUES) // 2
        half_of[perm] = np.arange(N_CLIQUES) % 2

        # pair-groups: distinct (block, pair entry)
        pkey = g_blk * NPAIR + pair_of[g_cq]
        upg, g2pg = np.unique(pkey, return_inverse=True)
        pg_blk = upg // NPAIR
        pg_p = upg % NPAIR
        cnt[c] = np.bincount(pg_blk, minlength=NBLK)
        xcP = np.ascontiguousarray(xc_bf[perm]).reshape(NPAIR, 2 * D)
        per_core.append(
            (g_cq, gid, win, half_of, g2pg, pg_blk, pg_p, xcP)
        )

    # Position order: position j holds each core's j-th largest block.
    orders = [np.argsort(-cnt[c], kind="stable") for c in range(N_CORES)]
    L = np.stack([cnt[c][orders[c]] for c in range(N_CORES)]).max(axis=0)
    sch = _sched_stream(L)

    # Pass 2 (per core): slots, idx, one-hots.
    E = sch["E"]
    n_slots = sch["ntile"] * 128
    cols = sch["cols"]
    j_m = np.array([j for j, _ in cols])
    t_m = np.array([t for _, t in cols])
    sl = t_m[:, None] * 128 + np.arange(128)[None, :]
    mask = (sl >= E[j_m][:, None]) & (sl < E[j_m + 1][:, None])

    in_maps = []
    for c in range(N_CORES):
        g_cq, gid, win, half_of, g2pg, pg_blk, pg_p, xcP = per_core[c]
        offs = np.concatenate([[0], np.cumsum(np.bincount(pg_blk, minlength=NBLK))])
        rank = np.arange(len(pg_blk)) - offs[pg_blk]
        inv_pos = np.empty(NBLK, dtype=np.int64)
        inv_pos[orders[c]] = np.arange(NBLK)
        slot = E[inv_pos[pg_blk]] + rank
        idx = np.zeros(n_slots, dtype=np.int16)
        idx[slot] = pg_p.astype(np.int16)

        # per-edge slot + half -> multi-hot [slot, half, win]
        e_slot = slot[g2pg[gid]]
        e_half = half_of[g_cq[gid]]
        oh_full = np.zeros((n_slots, 2, BLK), dtype=np.float32)
        np.add.at(oh_full, (e_slot, e_half, win), 1.0)
        oh4 = oh_full.reshape(sch["ntile"], 128, 2, BLK)
        ohcols = np.where(mask[:, :, None, None], oh4[t_m], 0.0)
        # device layout [128, 2*ncols, 128]: col 2m=even half, 2m+1=odd
        ohcols = np.ascontiguousarray(
            ohcols.transpose(1, 0, 2, 3).reshape(128, 2 * len(cols), BLK)
        ).astype(_NP_FP8)

        # invc / cnt laid out position-major (column j = block orders[c][j])
        inv_blk = np.zeros(NPAD, dtype=np.float32)
        inv_blk[:NPC] = inv_cnt[c * NPC : (c + 1) * NPC]
        inv_t = inv_blk.reshape(NBLK, BLK)[orders[c]].T

        in_maps.append(
            {
                "idx": _wrap(idx),
                "oh": ohcols,
                "xcP": xcP,
                "invc": np.ascontiguousarray(inv_t),
            }
        )

    shared = {}
    sched = tuple(int(t) for t in L)
    return in_maps, shared, sched, orders


# ----------------------------------------------------------------------------
# Kernel builder
# ----------------------------------------------------------------------------

def _build(sched):
    L = np.array(sched)
    sch = _sched_stream(L)
    assert 2 * max(m1 - m0 for m0, m1 in sch["call_cols"]) <= OHW

    from concourse.bacc import Bacc

    nc = Bacc(None, num_swdge_queues=NQ)
    xcP = nc.declare_dram_parameter("xcP", [NPAIR, 2 * D], _BF16, isOutput=False)
    idx = nc.declare_dram_parameter(
        "idx", [128, sch["ntile"] * 8], mybir.dt.int16, isOutput=False
    )
    oh = nc.declare_dram_parameter(
        "oh", [128, 2 * len(sch["cols"]), 128], _FP8, isOutput=False
    )
    invc = nc.declare_dram_parameter("invc", [128, NBLK], _F32, isOutput=False)
    wt = nc.declare_dram_parameter("wt", [128, 128], _BF16, isOutput=False)
    brow = nc.declare_dram_parameter("brow", [128, 128], _F32, isOutput=False)
    out = nc.declare_dram_parameter("out", [NPAD, D], _F32, isOutput=True)

    from contextlib import ExitStack

    with PatchedTileContext(nc) as tc, ExitStack() as ctx:
        const = ctx.enter_context(tc.tile_pool(name="const", bufs=1))
        gpool = ctx.enter_context(tc.tile_pool(name="g", bufs=2))
        opool = ctx.enter_context(tc.tile_pool(name="o", bufs=2))
        sb = ctx.enter_context(tc.tile_pool(name="sb", bufs=3))
        ps = ctx.enter_context(tc.tile_pool(name="ps", bufs=6, space="PSUM"))
        psl = ctx.enter_context(tc.tile_pool(name="psl", bufs=2, space="PSUM"))

        # warm-up: load the dma_gather ucode IRAM during the ramp
        widx_t = const.tile([128, 8], mybir.dt.int16)
        nc.vector.memset(widx_t[:], 0)
        wg_t = const.tile([128, 1, 2 * D], _BF16)
        nc.gpsimd.dma_gather(
            wg_t[:], xcP[:], widx_t[:], 128, 128, 2 * D,
            single_packet=False, queue_num=0,
        )
        idx_t = const.tile([128, sch["ntile"] * 8], mybir.dt.int16)
        calls = sch["calls"]
        h0 = (calls[0][0] + calls[0][1]) * 8
        kk = min(7, len(calls) - 1)
        h1 = (calls[kk][0] + calls[kk][1]) * 8
        nc.sync.dma_start(idx_t[:, :h0], idx[:, :h0])
        nc.sync.dma_start(idx_t[:, h0:h1], idx[:, h0:h1])
        nc.sync.dma_start(idx_t[:, h1:], idx[:, h1:])
        invc_t = const.tile([128, NBLK], _F32)
        nc.sync.dma_start(invc_t[:], invc[:])
        wt_t = const.tile([128, 128], _BF16)
        nc.sync.dma_start(wt_t[:], wt[:])
        brow_t = const.tile([128, 128], _F32)
        nc.sync.dma_start(brow_t[:], brow[:])

        call_tiles = {}   # call idx -> (gathered tile, onehot tile)
        emitted = [0]

        def start_pos(i):
            m0, m1 = sch["call_cols"][i]
            return sch["cols"][m0][0] if m0 < m1 else NBLK

        qload = [0] * NQ

        def emit_calls(up_to_pos):
            while emitted[0] < len(sch["calls"]) and start_pos(emitted[0]) <= up_to_pos:
                k = emitted[0]
                # least-loaded queue (by descriptor count): the conveyor ends
                # when the LAST ring drains, so keep rings balanced
                qi = min(range(NQ), key=lambda q: (qload[q], q))
                qload[qi] += sch["calls"][k][1]
                t0, nt = sch["calls"][k]
                m0, m1 = sch["call_cols"][k]
                w = m1 - m0
                g_t = gpool.tile([128, NT, 2 * D], _BF16, tag=f"g{qi}")
                nc.gpsimd.dma_gather(
                    g_t[:, :nt, :],
                    xcP[:],
                    idx_t[:, t0 * 8 : (t0 + nt) * 8],
                    nt * 128,
                    nt * 128,
                    2 * D,
                    single_packet=False,
                    queue_num=qi,
                )
                oh_t = opool.tile([128, OHW, 128], _FP8, tag=f"o{qi}")
                nc.sync.dma_start(oh_t[:, : 2 * w, :], oh[:, 2 * m0 : 2 * m1, :])
                call_tiles[k] = (g_t, oh_t)
                emitted[0] += 1

        for j in range(NBLK):
            emit_calls(j)
            c0, ncols = sch["pos_cols"][j]
            mms = [(m, sch["cols"][m][1], h) for m in range(c0, c0 + ncols)
                   for h in (0, 1)]
            accum = ps.tile([128, 128], _F32, tag="acc")
            for i, (m, t, h) in enumerate(mms):
                ci = sch["tile_call"][t]
                g_t, oh_t = call_tiles[ci]
                g_slot = t - sch["calls"][ci][0]
                oh_slot = 2 * (m - sch["call_cols"][ci][0]) + h
                nc.tensor.matmul(
                    out=accum[:],
                    lhsT=g_t[:, g_slot, h * D : (h + 1) * D],
                    rhs=oh_t[:, oh_slot, :],
                    start=(i == 0),
                    stop=(i == len(mms) - 1),
                )
            # accum[f, n] is summed.T -- exactly the lhsT the Linear wants.
            acc_sb = sb.tile([128, 128], _BF16, tag="accsb")
            nc.scalar.activation(
                acc_sb[:], accum[:], mybir.ActivationFunctionType.Copy
            )
            # lin[n, o] = summed[n, :] @ W.T + max(cnt[n],1)*b[o]; the rank-1
            # count*bias term makes the later 1/max(cnt,1) scale yield "+b".
            lin = psl.tile([128, 128], _F32, tag="lin")
            nc.tensor.matmul(
                out=lin[:], lhsT=acc_sb[:], rhs=wt_t[:], start=True, stop=True
            )
            # out[n, o] = lin[n, o] / max(count[n], 1)
            sc = sb.tile([128, 128], _F32, tag="sc")
            nc.scalar.activation(
                sc[:],
                lin[:],
                mybir.ActivationFunctionType.Copy,
                scale=invc_t[:, j : j + 1],
            )
            ob = sb.tile([128, 128], _F32, tag="ob")
            nc.vector.tensor_tensor(
                out=ob[:],
                in0=sc[:],
                in1=brow_t[:],
                op=mybir.AluOpType.add,
            )
            nc.sync.dma_start(out[j * 128 : (j + 1) * 128, :], ob[:])

    nc.finalize()
    return nc


_BUILD_CACHE = {}


def kernel(x, x_clique, node2clique_index, W, b, _trace=False, _tmpdir=None):
    in_maps, shared, sched, orders = _prepare(x_clique, node2clique_index)

    shared["wt"] = np.ascontiguousarray(
        np.asarray(W, dtype=np.float32).T
    ).astype(_NP_BF16)
    shared["brow"] = np.ascontiguousarray(
        np.tile(np.asarray(b, dtype=np.float32)[None, :], (128, 1))
    )

    if sched not in _BUILD_CACHE:
        _BUILD_CACHE[sched] = _build(sched)
    nc = _BUILD_CACHE[sched]

    full_maps = [dict(m, **shared) for m in in_maps]
    kwargs = {}
    if _trace:
        kwargs = dict(trace=True, tmpdir=_tmpdir)
    res = run_bass_kernel_spmd(nc, full_maps, core_ids=list(range(N_CORES)), **kwargs)

    # un-permute: device position j on core c holds block orders[c][j]
    out = np.empty((N_NODES, D), dtype=np.float32)
    for c in range(N_CORES):
        o = np.asarray(res.results[c]["out"], dtype=np.float32)
        arranged = np.empty((NBLK, BLK, D), dtype=np.float32)
        arranged[orders[c]] = o.reshape(NBLK, BLK, D)
        out[c * NPC : (c + 1) * NPC] = arranged.reshape(NPAD, D)[:NPC]
    if _trace:
        return out, res
    return out

